# revision 6
# baseline (speedup 1.0000x reference)
"""Multi-head attention (RoPE, causal) Trainium2 Bass kernel, 8-core SPMD. v4

Sharding: tensor-parallel over heads (2 heads/core) for QKV+attention.
Attention-output AllToAll is CHUNKED ([3,2,2,1] steps) with an interleaved
dest mapping (every attention step sends 64 tokens to each core), so
collectives overlap attention compute.  Output projection runs per chunk.

v4: software-pipelined main loop — phase1(t+1) is split into units (q-proj,
k-proj, v-proj, v-transpose) issued BETWEEN attention(t) score/AV blocks, so
the Activation engine (exp) always has scores queued while the PE does
projection work, instead of the two engines alternating idle phases.
PE keep-warm matmuls bridge the final collective so the tail projection
doesn't pay the cold p-state penalty.

Layouts as v2/v3: feature-on-partition f32r q/k with matmul rope; exp->bf16
attn weights; bf16 v/stage/cat/Wo; PSUM rings: p1[2]=1 bank each (p_q/p_perm/
p_v/p_t/p_bc/po/warm), mm[2]=2 banks each (p_s), av[2]=1 bank each.
"""
import numpy as np
from contextlib import ExitStack

import concourse.bass as bass
import concourse.mybir as mybir
import concourse.tile as tile
from concourse.bass_utils import run_bass_kernel_spmd

N_CORES = 8
B, S, D, H, DK = 2, 2048, 1024, 16, 64
T = B * S                    # 4096 flat tokens, batch-major
TT = 512                     # token tile (phase 1 / q tiles)
KT = 128                     # k tile (scores partition dim)
NT = T // TT                 # 8 token tiles
CHUNKS = [(0, 1), (2, 3), (4, 5), (6, 7)]    # a2a chunking of attention steps
NCH = len(CHUNKS)
CHUNK_OF = {t: c for c, steps in enumerate(CHUNKS) for t in steps}
WARM_A = 12                  # keep-warm before proj(NCH-2)
AV_LAG = 3                   # AV accumulation lags scores/exp by this many blocks
WARM_B = 190                 # keep-warm bridging the last collective (256-wide)
F32 = mybir.dt.float32
F32R = mybir.dt.float32r
BF16 = mybir.dt.bfloat16
AF = mybir.ActivationFunctionType
SCALE = 1.0 / np.sqrt(DK)

_cache = {}


def _consts():
    inv_freq = 10000.0 ** (-(np.arange(0, DK, 2, dtype=np.float64) / DK))
    pos = np.arange(S, dtype=np.float64)
    ang = pos[:, None] * inv_freq[None, :]                 # [S, 32]
    cos = np.repeat(np.cos(ang), 2, axis=1).T              # [64, S]
    sin = np.repeat(np.sin(ang), 2, axis=1).T
    cosT = np.concatenate([cos, cos], 0).astype(np.float32)   # [128, S]
    sinT = np.concatenate([sin, sin], 0).astype(np.float32)
    # P2T = P.T blockdiag for 2 heads; (P v)[2i] = -v[2i+1], (P v)[2i+1] = v[2i]
    p = np.zeros((DK, DK), np.float32)
    for i in range(DK // 2):
        p[2 * i, 2 * i + 1] = -1.0
        p[2 * i + 1, 2 * i] = 1.0
    p2t = np.zeros((128, 128), np.float32)
    p2t[:DK, :DK] = p.T
    p2t[DK:, DK:] = p.T
    ident = np.eye(128, dtype=np.float32)
    ones64 = np.ones((1, DK), np.float32)
    return cosT, sinT, p2t, ident, ones64


def split_multi_waits(nc, max_waits=1):
    """This walrus build allows fewer sync-waits per instruction than Tile's
    final drain carries; hoist extras onto same-engine NOPs inserted before."""
    for fn in nc.m.functions:
        for blk in fn.blocks:
            insts = blk.instructions
            out = []
            for inst in insts:
                si = getattr(inst, "sync_info", None)
                waits = list(si.on_wait) if si is not None else []
                if len(waits) > max_waits:
                    extra, keep = waits[:-max_waits], waits[-max_waits:]
                    for j, w in enumerate(extra):
                        nop = mybir.InstNoOp(
                            name=f"{inst.name}-wsplit{j}", ins=[], outs=[]
                        )
                        nop.engine = inst.engine
                        nop.sync_info = mybir.SyncInfo(on_wait=[w], on_update=[])
                        out.append(nop)
                    inst.sync_info = mybir.SyncInfo(
                        on_wait=keep, on_update=list(si.on_update)
                    )
                out.append(inst)
            insts[:] = out


def build_nc(repeat=1):
    cosT_np, sinT_np, p2t_np, ident_np, ones64_np = _consts()

    nc = bass.Bass("TRN2", target_bir_lowering=False, debug=False,
                   num_devices=N_CORES)
    xT = nc.declare_dram_parameter("xT", [D, T], F32R, isOutput=False)
    wq = nc.declare_dram_parameter("wq", [D, 128], F32R, isOutput=False)
    wk = nc.declare_dram_parameter("wk", [D, 128], F32R, isOutput=False)
    wv = nc.declare_dram_parameter("wv", [D, 128], F32R, isOutput=False)
    wo = nc.declare_dram_parameter("wo", [D, D], BF16, isOutput=False)
    y = nc.declare_dram_parameter("y", [TT, D], F32, isOutput=True)

    c_cos = nc.inline_tensor(cosT_np, name="c_cos")
    c_sin = nc.inline_tensor(sinT_np, name="c_sin")
    c_p2t = nc.inline_tensor(p2t_np, name="c_p2t")
    c_id = nc.inline_tensor(ident_np, name="c_id")
    import ml_dtypes
    c_idb = nc.inline_tensor(ident_np.astype(ml_dtypes.bfloat16), name="c_idb")
    c_on = nc.inline_tensor(ones64_np, name="c_on")

    # a2a chunk buffers: [dest, feat128, step-in-chunk, 64 tok]
    a2a_ins = [[nc.dram_tensor(f"a2a_in{r}_{c}", [N_CORES, 128, len(st), 64], BF16)
                for c, st in enumerate(CHUNKS)] for r in range(repeat)]
    a2a_outs = [[nc.dram_tensor(f"a2a_out{r}_{c}", [N_CORES, 128, len(st), 64], BF16)
                 for c, st in enumerate(CHUNKS)] for r in range(repeat)]

    with tile.TileContext(nc) as tc, ExitStack() as ctx:
        cst = ctx.enter_context(tc.tile_pool(name="cst", bufs=1))
        stream = ctx.enter_context(tc.tile_pool(name="stream", bufs=2))
        persist = ctx.enter_context(tc.tile_pool(name="persist", bufs=1))
        tmp = ctx.enter_context(tc.tile_pool(name="tmp", bufs=3))
        attnp = ctx.enter_context(tc.tile_pool(name="attnp", bufs=4))
        outp = ctx.enter_context(tc.tile_pool(name="outp", bufs=3))
        catp = ctx.enter_context(tc.tile_pool(name="catp", bufs=2))
        ps = ctx.enter_context(tc.tile_pool(name="ps", bufs=2, space="PSUM"))
        psav = ctx.enter_context(tc.tile_pool(name="psav", bufs=2, space="PSUM"))

        # ---- constants + weights to SBUF ----
        cos_s = cst.tile([128, S], F32)
        sin_s = cst.tile([128, S], F32)
        p2t_s = cst.tile([128, 128], F32R)
        id_s = cst.tile([128, 128], F32R)
        id_b = cst.tile([128, 128], BF16)
        on_s = cst.tile([1, DK], F32R)
        wq_s = cst.tile([128, 8, 128], F32R)
        wk_s = cst.tile([128, 8, 128], F32R)
        wv_s = cst.tile([128, 8, 128], F32R)
        wo_s = cst.tile([128, 8, D], BF16)
        # weights on the gpsimd queue; constants on the scalar queue (parallel),
        # sin/cos split so step-0's slice lands fast
        wq_r = wq.ap().rearrange("(g p) m -> p g m", p=128)
        nc.gpsimd.dma_start(out=wq_s[:, 0, :], in_=wq_r[:, 0, :])
        nc.gpsimd.dma_start(out=wq_s[:, 1:, :], in_=wq_r[:, 1:, :])
        nc.gpsimd.dma_start(
            out=wk_s[:], in_=wk.ap().rearrange("(g p) m -> p g m", p=128))
        nc.gpsimd.dma_start(
            out=wv_s[:], in_=wv.ap().rearrange("(g p) m -> p g m", p=128))
        nc.scalar.dma_start(out=p2t_s[:], in_=c_p2t.ap().bitcast(F32R))
        nc.scalar.dma_start(out=sin_s[:, 0:TT], in_=c_sin[:, 0:TT])
        nc.scalar.dma_start(out=cos_s[:, 0:TT], in_=c_cos[:, 0:TT])
        nc.scalar.dma_start(out=id_s[:], in_=c_id.ap().bitcast(F32R))
        nc.scalar.dma_start(out=id_b[:], in_=c_idb.ap())
        nc.scalar.dma_start(out=on_s[:], in_=c_on.ap().bitcast(F32R))

        # prewarm the Act engine's Exp table (real HW pays ~1.3us
        # ACT_TABLE_LOAD on first use; do it in the idle prologue, off the
        # step-0 critical path)
        wexp = cst.tile([1, DK], F32)
        nc.scalar.activation(wexp[:], on_s[:], AF.Exp, scale=0.001)

        # persistent activations
        qrot = persist.tile([128, T], F32R)
        krot = persist.tile([128, T], F32R)
        v_sb = persist.tile([128, T // KT, 130], BF16)  # [.., 0:64]+one | [.., 65:129]+one
        # cols 64 and 129 stay 1.0 (denominator ones)
        nc.vector.memset(v_sb[:].rearrange("p a b -> p (a b)"), 1.0)

        cur = {}
        xts = {}

        def xt_load(t):
            """Prefetch token tile t as 8 independent tiles (per-group deps;
            a consumer matmul only waits on its own group's DMA)."""
            xt = []
            for g in range(8):
                xg = stream.tile([128, TT], F32R, tag=f"xt{g}")
                nc.sync.dma_start(
                    out=xg[:],
                    in_=xT[g * 128:(g + 1) * 128, t * TT:(t + 1) * TT],
                )
                xt.append(xg)
            xts[t] = xt

        def phase1_units(t):
            """Thunks: q-proj+rope, k-proj+rope, v-proj, v-transpose for tile t."""
            pos = slice((t % (S // TT)) * TT, (t % (S // TT)) * TT + TT)
            tok = slice(t * TT, (t + 1) * TT)

            def qk_unit(w_sb, dst):
                xt = xts[t]
                p_q = ps.tile([128, TT], F32, tag="p1")
                for g in range(8):
                    nc.tensor.matmul(p_q[:], w_sb[:, g, :], xt[g][:],
                                     start=(g == 0), stop=(g == 7))
                qs = tmp.tile([128, TT], F32R, tag="qs")
                nc.vector.tensor_mul(qs[:], p_q[:], sin_s[:, pos])
                p_perm = ps.tile([128, TT], F32, tag="p1")
                nc.tensor.matmul(p_perm[:], p2t_s[:], qs[:], start=True, stop=True)
                qc = tmp.tile([128, TT], F32, tag="qc")
                nc.vector.tensor_mul(qc[:], p_q[:], cos_s[:, pos])
                nc.vector.tensor_add(dst[:, tok], qc[:], p_perm[:])

            def v_unit():
                xt = xts[t]
                p_v = ps.tile([128, TT], F32, tag="p1")
                for g in range(8):
                    nc.tensor.matmul(p_v[:], wv_s[:, g, :], xt[g][:],
                                     start=(g == 0), stop=(g == 7))
                vt = tmp.tile([128, TT], BF16, tag="vt")
                nc.vector.tensor_copy(vt[:], p_v[:])
                cur["vt", t] = vt

            def tr_unit():
                vt = cur.pop(("vt", t))
                for blk in range(TT // 128):
                    p_t = ps.tile([128, 128], BF16, tag="p1")
                    nc.tensor.transpose(p_t[:], vt[:, blk * 128:(blk + 1) * 128],
                                        id_b[:])
                    g = t * (TT // 128) + blk
                    vdst = v_sb[:, g, :].rearrange("p (a c) -> p a c", a=2)[:, :, 0:64]
                    nc.vector.tensor_copy(
                        vdst, p_t[:].rearrange("p (a c) -> p a c", a=2)
                    )

            return [(0.02, lambda: qk_unit(wq_s, qrot)),
                    (0.28, lambda: qk_unit(wk_s, krot)),
                    (0.55, v_unit), (0.8, tr_unit)]

        def attention(t, fillers=()):
            """q-tile of step t; interleave filler thunks between k-blocks."""
            b, J = t // 4, t % 4
            fillers = list(fillers)
            av0 = psav.tile([65, TT], F32, tag="av")
            av1 = psav.tile([65, TT], F32, tag="av")
            av = [av0, av1]
            nk = 4 * (J + 1)
            # filler f = (frac, thunk) fires after block int(frac*nk)
            slots = {}
            for f, (frac, _) in enumerate(fillers):
                slots.setdefault(min(nk - 1, int(frac * nk)), []).append(f)
            # bf16 shadows of tile t's last 128 q/k columns: the r=3
            # diagonal scores matmul has free dim 128, which fp32r runs at
            # 4 cy/row but bf16 at 1 cy/row
            qd = tmp.tile([128, KT], BF16, tag="qd")
            kd = tmp.tile([128, KT], BF16, tag="kd")
            nc.vector.tensor_copy(qd[:], qrot[:, (t + 1) * TT - KT:(t + 1) * TT])
            nc.vector.tensor_copy(kd[:], krot[:, (t + 1) * TT - KT:(t + 1) * TT])

            pend = []          # AV issue lags scores/exp by AV_LAG blocks

            def issue_av(i, at, qo, n):
                g = (b * S) // KT + i
                for h in range(2):
                    nc.tensor.matmul(
                        av[h][:, qo:TT], v_sb[:, g, 65 * h:65 * h + 65],
                        at[:, h, 0:n],
                        start=(i == 0), stop=(i == nk - 1),
                    )

            for i in range(nk):
                r = i - 4 * J          # >= 0 on diagonal blocks
                qo = KT * r if r > 0 else 0    # causal-narrowed q offset
                n = TT - qo
                p_s = ps.tile([128, 2, TT], F32, tag="mm")
                for h in range(2):
                    hp = slice(64 * h, 64 * h + 64)
                    if r == 3:     # diagonal 128-wide block: bf16 shadows
                        nc.tensor.matmul(p_s[:, h, 0:n], kd[hp, :], qd[hp, :],
                                         start=True, stop=True)
                    else:
                        nc.tensor.matmul(
                            p_s[:, h, 0:n],
                            krot[hp, b * S + i * KT: b * S + (i + 1) * KT],
                            qrot[hp, b * S + J * TT + qo: b * S + (J + 1) * TT],
                            start=True, stop=True,
                        )
                at = attnp.tile([128, 2, TT], BF16, tag="at")
                nc.scalar.activation(at[:, :, 0:n], p_s[:, :, 0:n], AF.Exp,
                                     scale=float(SCALE))
                if r >= 0:  # diagonal 128-block: zero where k > q
                    for h in range(2):
                        nc.gpsimd.affine_select(
                            out=at[:, h, 0:KT], in_=at[:, h, 0:KT],
                            compare_op=mybir.AluOpType.is_ge,
                            fill=0.0, base=0,
                            pattern=[[1, KT]], channel_multiplier=-1,
                        )
                pend.append((i, at, qo, n))
                if len(pend) > AV_LAG:
                    issue_av(*pend.pop(0))
                for f in slots.get(i, ()):
                    fillers[f][1]()
            for p in pend:
                issue_av(*p)
            # normalize by denominator row; stage bf16 for the a2a chunk
            stage = outp.tile([128, TT], BF16, tag="stage")
            for h in range(2):
                rec = tmp.tile([1, TT], F32R, tag="rec")
                with nc.allow_low_precision(reason="f32r recip for bcast matmul"):
                    nc.vector.reciprocal(rec[:], av[h][64:65, :])
                p_bc = ps.tile([64, TT], F32, tag="p1")
                nc.tensor.matmul(p_bc[:], on_s[:], rec[:], start=True, stop=True)
                hp = slice(64 * h, 64 * h + 64)
                nc.vector.tensor_copy(stage[hp, :], av[h][0:64, :])
                nc.vector.tensor_mul(stage[hp, :], stage[hp, :], p_bc[:])
            # scatter: 64 tokens to each dest core, step slot within its chunk
            c = CHUNK_OF[t]
            slot = t - CHUNKS[c][0]
            a2a_in = cur["a2a_in"][c]
            nc.sync.dma_start(
                out=a2a_in.ap().rearrange("d p h j -> p d h j")[:, :, slot, :],
                in_=stage[:].rearrange("p (d j) -> p d j", d=N_CORES),
            )
            cur["stage", t] = stage

        def proj_units(c):
            """Output projection thunks for chunk c (cat load + per-slab GEMMs)."""
            W = 64 * len(CHUNKS[c])
            row0 = 64 * CHUNKS[c][0]

            def load():
                cat = catp.tile([128, 8, W], BF16, tag="cat")
                nc.sync.dma_start(
                    out=cat[:],
                    in_=cur["a2a_out"][c].ap().rearrange("s p h j -> p s (h j)"),
                )
                cur["cat", c] = cat

            def slab(w0, n):
                cat = cur["cat", c]
                ww = min(128, W - w0)
                po = ps.tile([128, TT], F32, tag="p1")
                for s in range(8):
                    nc.tensor.matmul(po[0:ww, :], cat[:, s, w0:w0 + ww],
                                     wo_s[:, s, n * TT:(n + 1) * TT],
                                     start=(s == 0), stop=(s == 7))
                yt = outp.tile([128, TT], F32, tag="yt")
                nc.vector.tensor_copy(yt[0:ww, :], po[0:ww, :])
                nc.sync.dma_start(
                    out=y[row0 + w0:row0 + w0 + ww, n * TT:(n + 1) * TT],
                    in_=yt[0:ww, :],
                )

            units = [(0.45, load)]
            nslab = (W + 127) // 128 * 2
            for k, (w0, n) in enumerate(
                    (w0, n) for w0 in range(0, W, 128) for n in range(2)):
                units.append((0.62 + 0.36 * k / max(1, nslab - 1),
                              lambda w0=w0, n=n: slab(w0, n)))
            return units

        def proj(c):
            for _, u in proj_units(c):
                u()

        def warm(n_mm, anchor=None):
            """Keep the PE p-state hot while waiting on the last collective.
            `anchor` (an SBUF tile) adds a data dependency so the Tile
            scheduler cannot hoist these into earlier idle holes; 256-wide
            bf16 moving operand = 107ns each for fine-grained bridging."""
            for _ in range(n_mm):
                pw = ps.tile([128, TT], F32, tag="p1")
                if anchor is not None:
                    nc.tensor.matmul(pw[:, 0:256], id_b[:], anchor[:, 0:256],
                                     start=True, stop=True)
                else:
                    nc.tensor.matmul(pw[:], id_s[:], qrot[:, 0:TT],
                                     start=True, stop=True)

        def a2a(rep, c):
            nc.gpsimd.collective_compute(
                "AllToAll", mybir.AluOpType.bypass,
                replica_groups=[list(range(N_CORES))],
                ins=[a2a_ins[rep][c].ap().opt()],
                outs=[a2a_outs[rep][c].ap().opt()],
            )

        for rep in range(repeat):
            cur["a2a_in"] = a2a_ins[rep]
            cur["a2a_out"] = a2a_outs[rep]
            # prologue: tile 0 projected up front; PE ramped by warmup
            xt_load(0)
            if rep == 0:
                for _ in range(6):
                    pw = ps.tile([128, 128], F32, tag="p1")
                    nc.tensor.matmul(pw[:], wq_s[:, 0, :], wq_s[:, 0, :],
                                     start=True, stop=True)
            for _, u in phase1_units(0):
                u()
            if rep == 0:
                # bulky consts, needed from step-1 fillers / first proj on;
                # issued BEFORE any consumer, clock-gated off the prologue
                # DMA chain
                with tc.tile_wait_until(0.005):
                    nc.scalar.dma_start(out=sin_s[:, TT:], in_=c_sin[:, TT:])
                    nc.scalar.dma_start(out=cos_s[:, TT:], in_=c_cos[:, TT:])
            for t in range(8):
                if t == 1 and rep == 0:
                    with tc.tile_wait_until(0.018):
                        nc.sync.dma_start(
                            out=wo_s[:],
                            in_=wo.ap().rearrange("(g p) n -> p g n", p=128))
                fillers = []
                if t < 7:
                    xt_load(t + 1)
                    fillers += phase1_units(t + 1)
                if t == 5:
                    fillers += proj_units(0)

                attention(t, fillers)
                c = CHUNK_OF[t]
                if t == CHUNKS[c][-1]:          # chunk complete -> exchange
                    a2a(rep, c)
            proj(NCH - 3)
            proj(NCH - 2)
            warm(WARM_B, anchor=cur["stage", 7])
            proj(NCH - 1)

    split_multi_waits(nc)
    return nc


def _get_runner(repeat=1):
    """Build + jit once; returns f(in_maps) -> list of per-core output dicts."""
    key = ("runner", repeat)
    if key in _cache:
        return _cache[key]
    import jax
    import jax.numpy as jnp
    from jax.sharding import Mesh, PartitionSpec
    from jax.experimental.shard_map import shard_map
    from concourse import bass2jax, mybir as _mybir

    nc = build_nc(repeat=repeat)
    bass2jax.install_neuronx_cc_hook()

    in_names, out_names, out_avals, zero_outs = [], [], [], []
    for alloc in nc.m.functions[0].allocations:
        if not isinstance(_mybir.MemoryLocationSet, type) or not isinstance(
            alloc, _mybir.MemoryLocationSet
        ):
            continue
        name = alloc.memorylocations[0].name
        if alloc.kind == "ExternalInput":
            if name != "partition_id":
                in_names.append(name)
        elif alloc.kind == "ExternalOutput":
            out_names.append(name)
            shape = tuple(alloc.tensor_shape)
            dtype = _mybir.dt.np(alloc.dtype)
            out_avals.append(jax.core.ShapedArray(shape, dtype))
            zero_outs.append(np.zeros(shape, dtype))
    n_params = len(in_names)
    has_pid = nc.partition_id_tensor is not None
    all_names = in_names + out_names + (["partition_id"] if has_pid else [])

    def _body(*args):
        operands = list(args)
        if has_pid:
            operands.append(bass2jax.partition_id_tensor())
        outs = bass2jax._bass_exec_p.bind(
            *operands,
            out_avals=tuple(out_avals),
            in_names=tuple(all_names),
            out_names=tuple(out_names),
            lowering_input_output_aliases=(),
            sim_require_finite=True,
            sim_require_nnan=True,
            nc=nc,
        )
        return tuple(outs)

    devices = jax.devices()[:N_CORES]
    mesh = Mesh(np.asarray(devices), ("core",))
    n_outs = len(out_names)
    sharded = jax.jit(
        shard_map(
            _body, mesh=mesh,
            in_specs=(PartitionSpec("core"),) * (n_params + n_outs),
            out_specs=(PartitionSpec("core"),) * n_outs,
            check_rep=False,
        ),
        donate_argnums=tuple(range(n_params, n_params + n_outs)),
        keep_unused=True,
    )

    def make_bench(in_maps):
        from jax.sharding import NamedSharding
        sh = NamedSharding(mesh, PartitionSpec("core"))
        concat_in = [
            jax.device_put(
                np.concatenate([np.asarray(m[nm]) for m in in_maps], axis=0), sh)
            for nm in in_names
        ]
        zshapes = [(N_CORES * z.shape[0], *z.shape[1:]) for z in zero_outs]
        zdt = [z.dtype for z in zero_outs]
        mkz = jax.jit(
            lambda: tuple(jnp.zeros(s, d) for s, d in zip(zshapes, zdt)),
            out_shardings=tuple(sh for _ in zshapes),
        )

        def bench_once():
            zs = mkz()
            jax.block_until_ready(zs)
            t0 = __import__("time").perf_counter()
            out = sharded(*concat_in, *zs)
            jax.block_until_ready(out)
            return __import__("time").perf_counter() - t0

        return bench_once

    def run(in_maps):
        concat_in = [
            np.concatenate([np.asarray(m[nm]) for m in in_maps], axis=0)
            for nm in in_names
        ]
        concat_zeros = [
            np.zeros((N_CORES * z.shape[0], *z.shape[1:]), z.dtype)
            for z in zero_outs
        ]
        out_arrs = sharded(*concat_in, *concat_zeros)
        return [
            {nm: np.asarray(out_arrs[i]).reshape(N_CORES, *out_avals[i].shape)[c]
             for i, nm in enumerate(out_names)}
            for c in range(N_CORES)
        ]

    run.make_bench = make_bench
    _cache[key] = run
    return run


def _prep_in_maps(x, Wq, Wk, Wv, Wo):
    import ml_dtypes
    xT = np.ascontiguousarray(x.reshape(T, D).T)
    wo = np.ascontiguousarray(Wo).astype(ml_dtypes.bfloat16)
    in_maps = []
    for c in range(N_CORES):
        in_maps.append({
            "xT": xT,
            "wq": np.ascontiguousarray(np.concatenate([Wq[2 * c], Wq[2 * c + 1]], 1)),
            "wk": np.ascontiguousarray(np.concatenate([Wk[2 * c], Wk[2 * c + 1]], 1)),
            "wv": np.ascontiguousarray(np.concatenate([Wv[2 * c], Wv[2 * c + 1]], 1)),
            "wo": wo,
        })
    return in_maps


def _out_perm():
    """global token index for (core m, row r) of the per-core y [512, 1024].
    Row r holds token j=r%64 of attention step r//64, this core's 64-token
    dest slice."""
    m = np.arange(N_CORES)[:, None]
    r = np.arange(TT)[None, :]
    return (r // 64) * TT + m * 64 + (r % 64)     # [8, 512]


def _assemble_output(results):
    y_all = np.stack([r["y"] for r in results], axis=0)   # [8, 512, 1024]
    out = np.empty((T, D), np.float32)
    out[_out_perm().reshape(-1)] = y_all.reshape(N_CORES * TT, D)
    return out.reshape(B, S, D)


def kernel(x, Wq, Wk, Wv, Wo, repeat=1):
    x, Wq, Wk, Wv, Wo = (np.asarray(a, np.float32) for a in (x, Wq, Wk, Wv, Wo))
    run = _get_runner(repeat=repeat)
    results = run(_prep_in_maps(x, Wq, Wk, Wv, Wo))
    return _assemble_output(results)
